# revision 2
# baseline (speedup 1.0000x reference)
"""Distributed GCN reasoner kernel for 8 Trainium2 NeuronCores (Bass/Tile).

Strategy (graph/data parallel):
 - Core c owns dest nodes [c*NP,(c+1)*NP). Node state h kept feature-major
   in SBUF ([128 feats, NPpad nodes] fp32) across all 4 conv layers.
 - Per layer: hw = dis * (h @ W) computed locally (TensorE), cast fp16,
   written node-major to a DRAM piece; AllGather replicates the full
   [N,128] fp16 table to every core.
 - Edges (+self loops) of each core, grouped by (source block, dest window
   of 128), padded to 128-slot chunks with a uniform cross-core structure
   (single SPMD program). dma_gather pulls source rows (256B) from the
   table; a one-hot matmul (built on DVE via is_equal against an iota row)
   segment-sums each chunk into PSUM per window; flushes accumulate into an
   SBUF accumulator. Finalize: h' = act(acc*dis + h + b).
 - Classifier + precomputed dropout-mask multiply, output [N, 64] fp32.
"""
import sys
sys.path.insert(0, "/opt/trn_rl_repo")
import numpy as np

NC = 8
N_NODES = 100_000
N_EDGES = 1_600_000
GATHER_G = 1024


# ---------------------------------------------------------------- host prep
def build_graph_prep(N, edge_index, batch, query, G=4096):
    NP = N // NC
    NW = (NP + 127) // 128
    NPpad = NW * 128
    BS = 2 * NPpad
    assert BS <= 32768
    NB = NC // 2

    src = np.asarray(edge_index[0], dtype=np.int64)
    dst = np.asarray(edge_index[1], dtype=np.int64)
    loops = np.arange(N, dtype=np.int64)
    fs = np.concatenate([src, loops])
    fd = np.concatenate([dst, loops])

    deg = np.bincount(fd, minlength=N).astype(np.float32)
    dis = (1.0 / np.sqrt(np.maximum(deg, 1.0))).astype(np.float32)

    t_p = fs // NP
    t_row = (t_p % 2) * NPpad + (fs % NP)
    t_blk = t_p // 2

    order = np.argsort(fd, kind="stable")
    fs_row = t_row[order]
    fs_blk = t_blk[order]
    fd_s = fd[order]
    core_bounds = np.searchsorted(fd_s, np.arange(0, N + 1, NP))

    per_core = []
    counts = np.zeros((NC, NB, NW), np.int64)
    for c in range(NC):
        lo, hi = core_bounds[c], core_bounds[c + 1]
        cd = fd_s[lo:hi] - c * NP
        w_of = cd >> 7
        b_of = fs_blk[lo:hi]
        key = b_of * NW + w_of
        o2 = np.argsort(key, kind="stable")
        per_core.append((fs_row[lo:hi][o2], (cd - (w_of << 7))[o2], key[o2]))
        np.add.at(counts[c], (b_of, w_of), 1)

    nch = (counts.max(axis=0) + 127) // 128
    groups = []
    pos = 0
    for b in range(NB):
        for w in range(NW):
            if nch[b, w] > 0:
                groups.append((b, w, pos, int(nch[b, w])))
                pos += int(nch[b, w])
    CH = pos
    S = CH * 128

    calls = []
    Gch = G // 128
    for b in range(NB):
        sel = [g for g in groups if g[0] == b]
        if not sel:
            continue
        c0 = sel[0][2]
        c1 = sel[-1][2] + sel[-1][3]
        p = c0
        while p < c1:
            n = min(Gch, c1 - p)
            calls.append((b, p, n))
            p += n

    first_seen = {}
    chunk_meta = [None] * CH
    for (b, w, c0, n) in groups:
        fk = "copy" if w not in first_seen else "add"
        first_seen[w] = True
        for j in range(n):
            chunk_meta[c0 + j] = (w, j == 0, j == n - 1,
                                  fk if j == n - 1 else None)

    cores = []
    for c in range(NC):
        rows, drel, key = per_core[c]
        idx_all = np.zeros(S, np.int16)
        fd_all = np.full(S, -1.0, np.float16)
        uniq, starts, cnts = np.unique(key, return_index=True,
                                       return_counts=True)
        gp_pos = {(b, w): cs * 128 for (b, w, cs, _n) in groups}
        for k, st, cnt in zip(uniq, starts, cnts):
            b, w = int(k) // NW, int(k) % NW
            p0 = gp_pos[(b, w)]
            idx_all[p0:p0 + cnt] = rows[st:st + cnt].astype(np.int16)
            fd_all[p0:p0 + cnt] = drel[st:st + cnt].astype(np.float16)

        idx_w = np.zeros((128, S // 16), np.int16)
        iw = idx_all.reshape(S // 16, 16).T
        for r in range(8):
            idx_w[r * 16:(r + 1) * 16] = iw
        fd_w = fd_all.reshape(S // 128, 128).T.astype(np.float16)

        sl = slice(c * NP, (c + 1) * NP)
        tmpd = np.zeros(NPpad, np.float32)
        tmpd[:NP] = dis[sl]
        dis_nc = np.ascontiguousarray(tmpd.reshape(NW, 128).T)
        qT = np.zeros((128, NPpad), np.float32)
        qT[:, :NP] = np.asarray(query)[np.asarray(batch[sl])].T
        dis_rep = np.zeros((1, NPpad), np.float16)
        dis_rep[0, :NP] = dis[sl].astype(np.float16)
        dis_rep = np.broadcast_to(dis_rep, (128, NPpad)).copy()

        cores.append(dict(idx_w=idx_w, fd_w=fd_w, qT=qT, dis_nc=dis_nc,
                          dis_rep=dis_rep))

    meta = dict(NP=NP, NW=NW, NPpad=NPpad, BS=BS, NB=NB, S=S, CH=CH,
                groups=groups, calls=calls, chunk_meta=chunk_meta, G=G)
    return cores, meta


def core_inputs(inputs, cores, meta, mask_scale):
    NP, NW, NPpad = meta["NP"], meta["NW"], meta["NPpad"]
    x = np.asarray(inputs["x"], np.float32)
    Wall = np.concatenate([np.asarray(inputs[f"W{i}"], np.float32)
                           for i in range(4)], axis=1)
    bcol = np.stack([np.asarray(inputs[f"b{i}"], np.float32)
                     for i in range(4)], axis=1)
    Wc = np.asarray(inputs["Wc"], np.float32)
    bc_rep = np.broadcast_to(np.asarray(inputs["bc"], np.float32)[None, :],
                             (128, 64)).copy()
    iota_rep = np.tile(np.arange(128, dtype=np.float16)[None, :], (128, 1))

    maps = []
    for c in range(NC):
        sl = slice(c * NP, (c + 1) * NP)
        xT = np.zeros((128, NPpad), np.float32)
        xT[:, :NP] = x[sl].T
        tmp = np.zeros((NW * 128, 64), np.float16)
        tmp[:NP] = mask_scale[sl].astype(np.float16)
        mk = np.ascontiguousarray(tmp.reshape(NW, 128, 64).transpose(1, 0, 2))
        maps.append({
            "xT": xT, "qT": cores[c]["qT"],
            "dis16": cores[c]["dis_rep"], "disn": cores[c]["dis_nc"],
            "idx": cores[c]["idx_w"], "fdt": cores[c]["fd_w"],
            "Wall": Wall, "bcol": bcol, "Wc": Wc, "bcrep": bc_rep,
            "iota": iota_rep, "mask": mk.reshape(128, NW * 64),
        })
    return maps


# ------------------------------------------------------------- bass builder
def build_nc(meta):
    import concourse.bacc as bacc
    import concourse.tile as tile
    import concourse.mybir as mybir

    F32 = mybir.dt.float32
    F16 = mybir.dt.float16
    I16 = mybir.dt.int16
    OP = mybir.AluOpType
    AF = mybir.ActivationFunctionType

    NP, NW, NPpad = meta["NP"], meta["NW"], meta["NPpad"]
    BS, S = meta["BS"], meta["S"]
    calls, chunk_meta = meta["calls"], meta["chunk_meta"]
    NT = NC * NPpad

    nc = bacc.Bacc("TRN2", target_bir_lowering=False, debug=False,
                   num_devices=NC)
    d = lambda n, s, t: nc.dram_tensor(n, s, t, kind="ExternalInput").ap()
    xT_d = d("xT", [128, NPpad], F32)
    qT_d = d("qT", [128, NPpad], F32)
    dis16_d = d("dis16", [128, NPpad], F16)
    disn_d = d("disn", [128, NW], F32)
    idx_d = d("idx", [128, S // 16], I16)
    fdt_d = d("fdt", [128, S // 128], F16)
    Wall_d = d("Wall", [128, 512], F32)
    bcol_d = d("bcol", [128, 4], F32)
    Wc_d = d("Wc", [128, 64], F32)
    bcrep_d = d("bcrep", [128, 64], F32)
    iota_d = d("iota", [128, 128], F16)
    mask_d = d("mask", [128, NW * 64], F16)
    out_d = nc.dram_tensor("out", [NPpad, 64], F32, kind="ExternalOutput").ap()

    FIN_SL = 4
    with tile.TileContext(nc) as tc:
        with (
            tc.tile_pool(name="per", bufs=1) as per,
            tc.tile_pool(name="ldp", bufs=2) as ldp,
            tc.tile_pool(name="stg", bufs=2) as stg,
            tc.tile_pool(name="gat", bufs=2) as gat,
            tc.tile_pool(name="ohp", bufs=2) as ohp,
            tc.tile_pool(name="ixp", bufs=2) as ixp,
            tc.tile_pool(name="cls", bufs=2) as cls,
            tc.tile_pool(name="drm", bufs=1, space="DRAM") as drm,
            tc.tile_pool(name="pm", bufs=2, space="PSUM") as pm,
            tc.tile_pool(name="pw", bufs=4, space="PSUM") as pw,
            tc.tile_pool(name="pc", bufs=2, space="PSUM") as pc,
        ):
            piece = [drm.tile([NPpad, 128], F16, tag=f"piece{i}",
                              name=f"piece{i}") for i in range(4)]
            table = [drm.tile([NT, 128], F16, tag=f"table{i}",
                              name=f"table{i}", addr_space="Shared")
                     for i in range(4)]
            A = per.tile([128, NPpad], F32, tag="A")
            B = per.tile([128, NPpad], F32, tag="B")
            dis16 = per.tile([128, NPpad], F16, tag="dis16")
            disn = per.tile([128, NW], F32, tag="disn")
            Wall = per.tile([128, 512], F32, tag="Wall")
            bcol = per.tile([128, 4], F32, tag="bcol")
            Wc = per.tile([128, 64], F32, tag="Wc")
            bcrep = per.tile([128, 64], F32, tag="bcrep")
            iota = per.tile([128, 128], F16, tag="iota")

            for t, src in ((dis16, dis16_d), (disn, disn_d), (Wall, Wall_d),
                           (bcol, bcol_d), (Wc, Wc_d), (bcrep, bcrep_d),
                           (iota, iota_d)):
                nc.sync.dma_start(t[:], src)

            SLW = NPpad // FIN_SL
            H0_SL = 16
            H0W = NPpad // H0_SL if NPpad % H0_SL == 0 else None
            h0_bounds = ([(i * H0W, (i + 1) * H0W) for i in range(H0_SL)]
                         if H0W else [(i * 128, min((i + 1) * 128, NPpad))
                                      for i in range((NPpad + 127) // 128)])
            for (c0, c1) in h0_bounds:
                tx = ldp.tile([128, c1 - c0], F32, tag="ldx")
                tq = ldp.tile([128, c1 - c0], F32, tag="ldq")
                nc.sync.dma_start(tx[:], xT_d[:, c0:c1])
                nc.sync.dma_start(tq[:], qT_d[:, c0:c1])
                nc.vector.tensor_tensor(A[:, c0:c1], tx[:], tq[:], OP.mult)

            for li in range(4):
                h, acc = (A, B) if li % 2 == 0 else (B, A)
                pc_i, tb_i = piece[li], table[li]
                Wl = Wall[:, li * 128:(li + 1) * 128]
                piece_r = pc_i.rearrange("(a p) f -> p a f", p=128)

                w = 0
                while w < NW:
                    gs = min(8, NW - w)
                    st = stg.tile([128, gs, 128], F16, tag="stage")
                    for k in range(gs):
                        ps = pm.tile([128, 128], F32, tag="pm", name=f"pm_{li}_{w+k}")
                        nc.tensor.matmul(ps[:],
                                         h[:, (w + k) * 128:(w + k + 1) * 128],
                                         Wl, start=True, stop=True)
                        nc.vector.tensor_scalar(
                            st[:, k, :], ps[:], disn[:, w + k:w + k + 1], None,
                            op0=OP.mult)
                    nc.sync.dma_start(piece_r[:, w:w + gs, :], st[:])
                    w += gs

                nc.gpsimd.collective_compute(
                    "AllGather", OP.bypass,
                    replica_groups=[list(range(NC))],
                    ins=[pc_i], outs=[tb_i])

                cur_psum = {}
                for (b, ch0, nch) in calls:
                    ns = nch * 128
                    ix = ixp.tile([128, ns // 16], I16, tag="ix")
                    nc.sync.dma_start(ix[:], idx_d[:, ch0 * 8:(ch0 + nch) * 8])
                    ft = ixp.tile([128, nch], F16, tag="ft")
                    nc.sync.dma_start(ft[:], fdt_d[:, ch0:ch0 + nch])
                    g = gat.tile([128, nch, 128], F16, tag="g")
                    nc.gpsimd.dma_gather(
                        g[:], tb_i[b * BS:(b + 1) * BS, :], ix[:],
                        num_idxs=ns, num_idxs_reg=ns, elem_size=128)
                    oh = ohp.tile([128, nch, 128], F16, tag="oh")
                    j = 0
                    while j < nch:
                        js = min(8, nch - j)
                        fbc = ft[:, j:j + js].rearrange(
                            "p (j o) -> p j o", o=1).broadcast_to([128, js, 128])
                        ibc = iota[:].rearrange(
                            "p (j f) -> p j f", j=1).broadcast_to([128, js, 128])
                        nc.vector.tensor_tensor(oh[:, j:j + js, :], fbc, ibc,
                                                OP.is_equal)
                        j += js
                    for jl in range(nch):
                        w_, first, last, fk = chunk_meta[ch0 + jl]
                        if first:
                            cur_psum[w_] = pw.tile([128, 128], F32, tag="pw",
                                                   name=f"pw_{li}_{w_}_{b}")
                        nc.tensor.matmul(cur_psum[w_][:], g[:, jl, :],
                                         oh[:, jl, :], start=first, stop=last)
                        if last:
                            sl_ = acc[:, w_ * 128:(w_ + 1) * 128]
                            if fk == "copy":
                                nc.vector.tensor_copy(sl_, cur_psum[w_][:])
                            else:
                                nc.vector.tensor_tensor(sl_, sl_,
                                                        cur_psum[w_][:], OP.add)
                            del cur_psum[w_]

                for s in range(FIN_SL):
                    c0, c1 = s * SLW, (s + 1) * SLW
                    sl_ = acc[:, c0:c1]
                    nc.vector.tensor_tensor(sl_, sl_, dis16[:, c0:c1], OP.mult)
                    nc.vector.tensor_tensor(sl_, sl_, h[:, c0:c1], OP.add)
                    nc.scalar.activation(
                        sl_, sl_, AF.Relu if li < 3 else AF.Identity,
                        bias=bcol[:, li:li + 1])

            hfin = A
            out_r = out_d.rearrange("(a p) c -> p a c", p=128)
            mask_dr = mask_d.rearrange("p (a c) -> p a c", c=64)
            w = 0
            while w < NW:
                gs = min(8, NW - w)
                so = cls.tile([128, gs, 64], F32, tag="so")
                mk = cls.tile([128, gs, 64], F16, tag="mk")
                nc.sync.dma_start(mk[:], mask_dr[:, w:w + gs, :])
                for k in range(gs):
                    ps = pc.tile([128, 64], F32, tag="pc", name=f"pc_{w+k}")
                    nc.tensor.matmul(ps[:],
                                     hfin[:, (w + k) * 128:(w + k + 1) * 128],
                                     Wc[:], start=True, stop=True)
                    nc.vector.tensor_tensor(so[:, k, :], ps[:], bcrep[:], OP.add)
                    nc.vector.tensor_tensor(so[:, k, :], so[:, k, :],
                                            mk[:, k, :], OP.mult)
                nc.sync.dma_start(out_r[:, w:w + gs, :], so[:])
                w += gs

    nc.compile()
    return nc


# ------------------------------------------------------------------ runner
_CACHE = {}


def _dropout_mask():
    import jax
    cpu = jax.devices("cpu")[0]
    with jax.default_device(cpu):
        keep = np.asarray(jax.random.bernoulli(jax.random.key(42), 0.8,
                                               (N_NODES, 64)))
    return np.where(keep, np.float32(1.25), np.float32(0.0))


def kernel(**inputs):
    from concourse.bass_utils import run_bass_kernel_spmd

    inputs = {k: np.asarray(v) for k, v in inputs.items()}
    mask = _dropout_mask()
    cores, meta = build_graph_prep(N_NODES, inputs["edge_index"],
                                   inputs["batch"], inputs["query"], G=GATHER_G)
    maps = core_inputs(inputs, cores, meta, mask)
    if "nc" not in _CACHE:
        _CACHE["nc"] = build_nc(meta)
    nc = _CACHE["nc"]
    res = run_bass_kernel_spmd(nc, maps, core_ids=list(range(NC)))
    NP = meta["NP"]
    out = np.concatenate([res.results[c]["out"][:NP] for c in range(NC)], 0)
    return out.astype(np.float32)
